# revision 21
# baseline (speedup 1.0000x reference)
"""GNN ActorNetwork forward on 8 TRN2 NeuronCores (Bass/Tile).

Sharding: data-parallel by dst-node range (dag-aligned split of the sorted
`batch`). Host does index-only preprocessing (edge binning/sorting by dst,
x-row gather/transpose into the edge stream, segment metadata); every
floating-point op of the network runs on device.

Layouts: all large activations are feature-major and 4-quadrant striped:
tensor[32*q + f, c] = value(feature f, row q*Q + c). Weights become host
built block-diagonal [128,128] stacks so ONE plain matmul per 512-col chunk
serves all 4 quadrants. Edge phase:
  L1  relu1 = relu(W1bd @ xsrc_striped)                  (fm, striped)
  L2  relu2ones[4 windows] = relu(r1_colslice^T @ W2bd + b2)  (edge-major)
  scatter pre3deg[17, dst] += relu2ones_win^T @ onehot(dstloc) (PSUM accum,
      folds prep-L3 and in-degree; onehot via is_equal vs iota)
Then agg/proc/node/score fm chains; dag-level ops on small flat tensors.
"""

import os
import sys

if "/opt/trn_rl_repo" not in sys.path:
    sys.path.insert(0, "/opt/trn_rl_repo")


def _install_ntff_hook(so_path="/opt/axon/libaxon_pjrt.so"):
    """Recreate antenv.axon_hooks (missing in this image) so
    run_bass_kernel_spmd(trace=True) captures NTFF profiles via axon."""
    import contextlib
    import ctypes
    import types

    import antenv

    if "antenv.axon_hooks" in sys.modules:
        return
    try:
        lib = ctypes.CDLL(so_path)
    except OSError:
        return
    if not hasattr(lib, "axon_start_nrt_profile"):
        return
    lib.axon_start_nrt_profile.argtypes = [ctypes.POINTER(ctypes.c_int64),
                                           ctypes.c_size_t]
    lib.axon_start_nrt_profile.restype = ctypes.c_int64
    lib.axon_stop_nrt_profile.argtypes = [ctypes.c_char_p]
    lib.axon_stop_nrt_profile.restype = ctypes.c_int64

    @contextlib.contextmanager
    def _hook(output_dir, device_ids):
        import jax

        jax.devices()
        if device_ids:
            ids = (ctypes.c_int64 * len(device_ids))(*device_ids)
            rc = lib.axon_start_nrt_profile(ids, len(device_ids))
        else:
            rc = lib.axon_start_nrt_profile(None, 0)
        if rc != 0:
            raise RuntimeError(f"axon_start_nrt_profile rc={rc}")
        try:
            yield
        finally:
            n = lib.axon_stop_nrt_profile(str(output_dir).encode())
            print(f"profile: {n} file(s) written to {output_dir}")

    mod = types.ModuleType("antenv.axon_hooks")
    _state = {"hook": _hook}
    mod.set_axon_ntff_profile_hook = lambda h: _state.update(hook=h)
    mod.get_axon_ntff_profile_hook = lambda: _state["hook"]
    sys.modules["antenv.axon_hooks"] = mod
    antenv.axon_hooks = mod

    import concourse.bass_utils as _bu

    _bu.upload_artifacts = lambda tmpdir: ""

import numpy as np
import ml_dtypes

BF_W = {"p1", "p2", "a1", "c1x", "n1x", "s1x"}

from concourse import bass, mybir
import concourse.tile as tile
from concourse.tile import TileContext
from concourse.vector_clock import ScopedClock
from concourse.bass_utils import run_bass_kernel_spmd

BF = ml_dtypes.bfloat16
NCORES = 8
DE = 16
NWORK = 50
SEG_E = 16384           # edges per pipeline segment (4 quadrants x 4096)
QE = SEG_E // 4
SUB = 64                # dst nodes per scatter subtile
SUBGRP = 8              # subtiles per psum group (512 cols)

_neff_cache = {}


# ----------------------------------------------------------------------
# walrus workaround: split tail-drain sem waits (1 per CTRL instruction)
# ----------------------------------------------------------------------

def _drain_and_barrier(self, tick_clock, wait_clock):
    drain_inst = self.nc.sync.drain()
    wait_clock.add_sem_waits(
        drain_inst.ins, ScopedClock({None: tick_clock.global_clock}))
    si = drain_inst.ins.sync_info
    waits = list(si.on_wait) if si and si.on_wait else []
    if len(waits) > 1:
        si.on_wait = waits[:1]
        for i in range(1, len(waits)):
            extra = self.nc.sync.drain()
            extra.ins.sync_info = mybir.SyncInfo(on_wait=[waits[i]],
                                                 on_update=[])
    self.nc.all_engine_barrier()
    assert self.sems is not None
    popped = self.nc._tile_sem_poison_stack.pop()
    assert popped is self._sem_poison
    self.nc.clear_and_free_semaphores(list(self.sems.allocated().values()))
    self.nc.all_engine_barrier()


tile.TileContext._drain_and_barrier = _drain_and_barrier

_MAXW = 1
_orig_lower = tile.TileContext._lower_ordered_insts


def _lower_with_wait_split(self, ordered):
    nc = self.nc
    for bb_name, insts in ordered.items():
        out = []
        for inst in insts:
            si = getattr(inst, "sync_info", None)
            if si is not None and si.on_wait and len(si.on_wait) > _MAXW:
                waits = list(si.on_wait)
                eng = inst.engine
                for i in range(_MAXW, len(waits), _MAXW):
                    nop = mybir.InstEventSemaphore(
                        name=nc.get_next_instruction_name())
                    nop.engine = eng
                    nop.sync_info = mybir.SyncInfo(
                        on_wait=waits[i:i + _MAXW], on_update=[])
                    nc.register_instruction(nop, overwrite=True)
                    out.append(nop)
                si.on_wait = waits[:_MAXW]
            out.append(inst)
        insts[:] = out
    return _orig_lower(self, ordered)


tile.TileContext._lower_ordered_insts = _lower_with_wait_split


# ----------------------------------------------------------------------
# host-side index preprocessing
# ----------------------------------------------------------------------

def _prep(x, edge_index, batch, ptr, num_dags_per_obs):
    N = x.shape[0]
    G = ptr.shape[0] - 1
    B = num_dags_per_obs.shape[0]

    obs_indptr = np.concatenate([[0], np.cumsum(num_dags_per_obs)]).astype(
        np.int64)
    seg_starts = np.searchsorted(batch, np.arange(G + 1))
    # node index where each obs starts (obs o owns dags obs_indptr[o]:[o+1])
    ostart = seg_starts[np.minimum(obs_indptr, G)]
    cuts = [0]
    for c in range(1, NCORES):
        tgt = c * N // NCORES
        i = np.searchsorted(ostart, tgt)
        lo = int(ostart[i - 1]) if i > 0 else 0
        hi = int(ostart[i]) if i < len(ostart) else N
        cut = lo if tgt - lo <= hi - tgt else hi
        cuts.append(int(max(cut, cuts[-1])))
    cuts.append(N)
    cuts = np.array(cuts, np.int64)
    NMAX = int(np.diff(cuts).max())
    NMAX = ((NMAX + 2047) // 2048) * 2048
    NQ = NMAX // 4

    dag_core = np.searchsorted(cuts[1:], seg_starts[:G], side="right")
    dcuts = np.searchsorted(dag_core, np.arange(NCORES + 1))
    NDMAX = int(max(1, np.diff(dcuts).max()))

    src, dst = edge_index[0].astype(np.int64), edge_index[1].astype(np.int64)
    core_of_dst = np.searchsorted(cuts[1:], dst, side="right")

    cores = []
    nsub = NMAX // SUB
    percore = []
    for c in range(NCORES):
        lo, hi = int(cuts[c]), int(cuts[c + 1])
        n_c = hi - lo
        sel = np.nonzero(core_of_dst == c)[0]
        dl = dst[sel] - lo
        order = np.argsort(dl, kind="stable")
        sel, dl = sel[order], dl[order]
        sub_id = dl >> 6
        cnt = np.bincount(sub_id, minlength=nsub)
        percore.append((lo, hi, n_c, sel, dl, sub_id, cnt))
    wcnt = np.zeros(nsub, np.int64)
    for (lo, hi, n_c, sel, dl, sub_id, cnt) in percore:
        wcnt = np.maximum(wcnt, (cnt + 127) // 128)
    EP = int(wcnt.sum() * 128)
    pos0 = np.concatenate([[0], np.cumsum(wcnt * 128)])[:-1]
    wsub = np.repeat(np.arange(nsub), wcnt)
    wfirst = np.ones(len(wsub), bool)
    wfirst[1:] = wsub[1:] != wsub[:-1]
    wlast = np.ones(len(wsub), bool)
    wlast[:-1] = wsub[1:] != wsub[:-1]

    ref_segs = None
    ref_osegs = None
    for c in range(NCORES):
        (lo, hi, n_c, sel, dl, sub_id, cnt) = percore[c]
        within = np.arange(len(sel)) - \
            np.concatenate([[0], np.cumsum(cnt)])[:-1][sub_id]
        pos = pos0[sub_id] + within
        src_p = np.zeros(EP, np.int64)
        dloc_p = np.full(EP, -1.0, np.float32)
        src_p[pos] = src[sel]
        dloc_p[pos] = (dl - (sub_id << 6)).astype(np.float32)

        d0, d1 = int(dcuts[c]), int(dcuts[c + 1])
        segs = []
        for g in range(d0, d1):
            ss = max(int(seg_starts[g]) - lo, 0)
            e = min(int(seg_starts[g + 1]) - lo, n_c)
            if e > ss:
                segs.append((g - d0, ss, e - ss))
        osegs = []
        for o in range(B):
            a, b = int(obs_indptr[o]), int(obs_indptr[o + 1])
            a2, b2 = max(a, d0), min(b, d1)
            if b2 > a2:
                osegs.append((o, a2 - d0, b2 - a2, max(b - a, 1)))
        if n_c > 0:
            if ref_segs is None:
                ref_segs, ref_osegs = segs, osegs
            else:
                assert segs == ref_segs, (
                    "cores have mismatched dag structure; this kernel "
                    "requires an identical per-core dag layout")
                assert [t[1:] for t in osegs] == \
                    [t[1:] for t in ref_osegs]
        cores.append(dict(lo=lo, n=n_c, EP=EP, src_p=src_p, dloc_p=dloc_p,
                          wsub=wsub, wfirst=wfirst, wlast=wlast,
                          d0=d0, nd=d1 - d0, segs=ref_segs or [],
                          osegs=ref_osegs or []))
    for c in cores:
        c["segs"] = ref_segs or []
        c["osegs"] = ref_osegs or []

    EMAX = max(((EP + SEG_E - 1) // SEG_E) * SEG_E, SEG_E)
    for c in cores:
        pad = EMAX - EP
        c["src_p"] = np.concatenate([c["src_p"], np.zeros(pad, np.int64)])
        c["dloc_p"] = np.concatenate(
            [c["dloc_p"], np.full(pad, -1.0, np.float32)])
        c["nwin_real"] = EP // 128

    gosegs = [(o, int(obs_indptr[o]),
               int(min(obs_indptr[o + 1], G) - min(obs_indptr[o], G)))
              for o in range(B)]
    return dict(N=N, G=G, B=B, NMAX=NMAX, NQ=NQ, NDMAX=NDMAX, EMAX=EMAX,
                cores=cores, obs_indptr=obs_indptr, gosegs=gosegs)


def _bd4(W, K=32, M=32):
    """block-diag 4-stack of W [k,m] -> [128,128] (quadrant q at rows
    32q..32q+k, cols 32q..32q+m)."""
    k, m = W.shape
    out = np.zeros((128, 128), np.float32)
    for q in range(4):
        out[32 * q:32 * q + k, 32 * q:32 * q + m] = W
    return out


def _stack4(v):
    out = np.zeros((128, 1), np.float32)
    for q in range(4):
        out[32 * q:32 * q + len(v), 0] = v
    return out


def _prep_weights(params):
    P = {k: [(np.asarray(W, np.float32), np.asarray(b, np.float32))
             for W, b in v] for k, v in params.items()}
    w, b = {}, {}
    W1, b1 = P["prep"][0]
    w["p1"] = _bd4(np.vstack([W1, b1[None]]))          # [7,32] aug
    W2, b2 = P["prep"][1]
    w2 = np.zeros((32, 17), np.float32)
    w2[:, :16] = W2
    p2 = np.zeros((128, 68), np.float32)
    for q in range(4):
        p2[32 * q:32 * q + 32, 17 * q:17 * q + 17] = w2
    w["p2"] = p2
    b2r = np.zeros(17, np.float32)
    b2r[:16] = b2
    b2r[16] = 1.0                                      # ones column
    w["b2row"] = b2r
    W3, b3 = P["prep"][2]
    Wa1, ba1 = P["agg"][0]
    f = np.zeros((17, 32), np.float32)
    f[:16] = W3 @ Wa1
    f[16] = b3 @ Wa1
    w["a1"], b["a1"] = _bd4(f), _stack4(ba1)
    for nm, key in (("agg", "a"), ("proc", "c"), ("node", "n")):
        ls = P[nm]
        if nm != "agg":
            Wl, bl = ls[0]
            w[key + "1x"] = _bd4(Wl[:6])
            w[key + "1a"] = _bd4(Wl[6:])
            b[key + "1"] = _stack4(bl)
        w[key + "2"], b[key + "2"] = _bd4(ls[1][0]), _stack4(ls[1][1])
        w[key + "3"], b[key + "3"] = _bd4(ls[2][0]), _stack4(ls[2][1])
    (Ws1, bs1), (Ws2, bs2), (Ws3, bs3) = P["node_score"]
    w["s1x"], w["s1n"] = _bd4(Ws1[:6]), _bd4(Ws1[6:22])
    w["s1d"], w["s1g"] = Ws1[22:38], Ws1[38:54]        # flat [16,32]
    b["s1"] = _stack4(bs1)
    w["s2"], b["s2"] = _bd4(Ws2), _stack4(bs2)
    w["s3"] = _bd4(Ws3, M=1)                           # col 32q
    w["s3b"] = float(bs3[0])
    for i, (Wd, bd_) in enumerate(P["dag"]):
        w[f"d{i + 1}"], b[f"d{i + 1}"] = Wd, bd_       # flat (small)
    (Wg1, bg1), (Wg2, bg2), (Wg3, bg3) = P["dag_score"]
    w["g1f"], w["g1e"], w["g1g"], w["g1w"] = Wg1[:3], Wg1[3:19], Wg1[19:35], \
        Wg1[35:36]
    b["g1"] = bg1
    w["g2"], b["g2"] = Wg2, bg2
    w["g3"], w["g3b"] = Wg3, float(bg3[0])
    return w, b


# ----------------------------------------------------------------------
# device kernel
# ----------------------------------------------------------------------

def _build(meta, wsh, bsh, s3b, g3b, core0):
    bf = mybir.dt.bfloat16
    f32 = mybir.dt.float32
    Relu = mybir.ActivationFunctionType.Relu
    Copy = mybir.ActivationFunctionType.Copy
    add = mybir.AluOpType.add
    X = mybir.AxisListType.X
    NMAX, NQ, NDMAX, EMAX = meta["NMAX"], meta["NQ"], meta["NDMAX"], \
        meta["EMAX"]
    G, B = meta["G"], meta["B"]
    c = core0
    nwin_tot = EMAX // 128
    nseg = EMAX // SEG_E
    NDS = NDMAX * NWORK

    nc = bass.Bass("TRN2", target_bir_lowering=False, debug=False,
                   num_devices=NCORES)
    dp = lambda n, s, d=bf: nc.dram_tensor(n, s, d, kind="ExternalInput").ap()
    do = lambda n, s, d=f32: nc.dram_tensor(n, s, d,
                                            kind="ExternalOutput").ap()

    xsrc_d = dp("xsrc", [128, EMAX // 4])
    dloc_d = dp("dloc", [128, nwin_tot])
    xt_d = dp("xt", [128, NQ])
    dfeat_d = dp("dfeat", [3, NDMAX])
    iota_d = dp("iota64", [128, 64])
    wk_d = dp("wk", [1, NWORK])
    ptrT_d = dp("ptrT", [1, G + 1], f32)
    ndpoT_d = dp("ndpoT", [B, 1])
    tri_d = dp("tri", [B, B + 1])
    wd = {}
    for k, v in wsh.items():
        if k in ("p1", "p2", "a1", "c1x", "n1x", "s1x"):
            dt_ = bf
        elif k in ("a2", "a3", "c1a", "c2", "c3", "n1a", "n2", "n3", "s1n", "s2", "s3"):
            dt_ = mybir.dt.float32r
        else:
            dt_ = f32
        wd[k] = dp("w_" + k, list(v), dt_)
    bd = {k: dp("b_" + k, list(v), f32) for k, v in bsh.items()}

    ns_d = do("ns", [128, NQ])
    ds_d = do("ds", [1, NDS])
    nno_d = do("nno", [1, B])
    oip_d = do("oip", [B + 1, 1])

    CH = min(2048, NQ)
    f32r = mybir.dt.float32r

    with TileContext(nc) as tc:
        with tc.tile_pool(name="const", bufs=1) as cp:
            W = {}
            for k, v in wsh.items():
                if k in ("p1", "p2", "a1", "c1x", "n1x", "s1x"):
                    dt_ = bf
                elif k in ("a2", "a3", "c1a", "c2", "c3", "n1a", "n2", "n3", "s1n", "s2", "s3"):
                    dt_ = mybir.dt.float32r
                else:
                    dt_ = f32
                W[k] = cp.tile(list(v), dt_, name="w_" + k)
                nc.sync.dma_start(out=W[k][:, :], in_=wd[k][:, :])
            Bt = {}
            for k, v in bsh.items():
                Bt[k] = cp.tile(list(v), f32, name="b_" + k)
                nc.sync.dma_start(out=Bt[k][:, :], in_=bd[k][:, :])
            iota = cp.tile([128, 64], bf)
            nc.sync.dma_start(out=iota[:, :], in_=iota_d[:, :])
            xt = cp.tile([128, NQ], bf)
            nc.sync.dma_start(out=xt[:, :], in_=xt_d[:, :])
            b2row = cp.tile([128, 17], f32)
            nc.sync.dma_start(out=b2row[:, :], in_=bd["b2row"][:, :])

            npo_cm = tc.tile_pool(name="npo", bufs=1)
            npo = npo_cm.__enter__()
            nemb = npo.tile([128, NQ], f32r, name="nemb")
            qd = npo.tile([128, NDMAX], f32, name="qd")
            nc.vector.memset(qd[:, :], 0.0)

            p1_cm = tc.tile_pool(name="p1", bufs=1)
            p1 = p1_cm.__enter__()
            dloc = p1.tile([128, nwin_tot], bf, name="dloc")
            nc.sync.dma_start(out=dloc[:, :], in_=dloc_d[:, :])
            pre3 = p1.tile([128, NQ], bf, name="pre3")
            nc.vector.memset(pre3[:, :], 0.0)

            # ================= edge phase =================
            with (
                tc.tile_pool(name="eph", bufs=2) as ep,
                tc.tile_pool(name="ep1", bufs=2, space="PSUM") as pp1,
                tc.tile_pool(name="ep2", bufs=2, space="PSUM") as pp2,
                tc.tile_pool(name="eps", bufs=2, space="PSUM") as pps,
            ):
                sub_ps = None
                sub_grp = -1

                def flush_group():
                    q, g = divmod(sub_grp, NQ // (SUBGRP * SUB))
                    g0 = g * SUBGRP * SUB
                    nc.vector.tensor_copy(
                        out=pre3[32 * q:32 * q + 17, g0:g0 + SUBGRP * SUB],
                        in_=sub_ps[:, :])

                for sgi in range(nseg):
                    xs = ep.tile([128, QE], bf, name="xs", tag="xs")
                    nc.sync.dma_start(
                        out=xs[:, :], in_=xsrc_d[:, sgi * QE:(sgi + 1) * QE])
                    r1 = ep.tile([128, QE], bf, name="r1", tag="r1")
                    for k in range(QE // 512):
                        ps = pp1.tile([128, 512], f32, name="l1", tag="l1")
                        nc.tensor.matmul(ps[:, :], W["p1"][:, :],
                                         xs[:, k * 512:(k + 1) * 512],
                                         start=True, stop=True)
                        nc.scalar.activation(r1[:, k * 512:(k + 1) * 512],
                                             ps[:, :], Relu)
                    r2 = ep.tile([128, 32 * 68], bf, name="r2", tag="r2", bufs=1)
                    for j in range(32):
                        ps2 = pp2.tile([128, 68], f32, name="l2", tag="l2")
                        nc.tensor.matmul(ps2[:, :],
                                         r1[:, j * 128:(j + 1) * 128],
                                         W["p2"][:, :], start=True, stop=True)
                        nc.vector.tensor_tensor(
                            out=ps2[:, :], in0=ps2[:, :],
                            in1=b2row[:, None, :].to_broadcast([128, 4, 17]),
                            op=add)
                        nc.scalar.activation(r2[:, j * 68:(j + 1) * 68],
                                             ps2[:, :], Relu)
                    oh = ep.tile([128, 128 * 64], bf, name="oh", tag="oh", bufs=1)
                    nc.vector.tensor_tensor(
                        out=oh[:, :],
                        in0=dloc[:, sgi * 128:(sgi + 1) * 128,
                                 None].to_broadcast([128, 128, 64]),
                        in1=iota[:, None, :].to_broadcast([128, 128, 64]),
                        op=mybir.AluOpType.is_equal)
                    for wl in range(128):
                        gw = sgi * 128 + wl
                        if gw >= c["nwin_real"]:
                            break
                        q, jw = divmod(wl, 32)
                        r2w = r2[:, jw * 68 + q * 17: jw * 68 + q * 17 + 17]
                        sub = int(c["wsub"][gw])
                        grp = sub // SUBGRP
                        if grp != sub_grp:
                            if sub_ps is not None:
                                flush_group()
                            sub_ps = pps.tile([17, SUBGRP * SUB], f32,
                                              name="sc", tag="sc")
                            sub_grp = grp
                        col = (sub % SUBGRP) * SUB
                        nc.tensor.matmul(
                            sub_ps[:, col:col + SUB], r2w,
                            oh[:, wl * 64:(wl + 1) * 64],
                            start=bool(c["wfirst"][gw]),
                            stop=bool(c["wlast"][gw]),
                            skip_group_check=True)
                if sub_ps is not None:
                    flush_group()

            # ================= node phase (pass 1) =================
            segs = c["segs"]

            with (
                tc.tile_pool(name="nsc", bufs=1) as nsc,
                tc.tile_pool(name="npp", bufs=2, space="PSUM") as npp,
            ):
                def lay(dst, doff, ops, bias, relu, extra=None, k0abs=0):
                    for k in range(0, CH, 512):
                        ps = npp.tile([128, 512], f32, name="ps", tag="ps")
                        n = len(ops)
                        for i, (Wi, Si, soff, rc) in enumerate(ops):
                            lh, rh = Wi[:, :], Si[:, soff + k:soff + k + 512]
                            nc.tensor.matmul(ps[:, :], lh, rh,
                                             start=(i == 0),
                                             stop=(i == n - 1))
                        if extra is not None:
                            extra(ps, k0abs + k, k0abs + k + 512)
                        o = dst[:, doff + k:doff + k + 512]
                        if relu:
                            nc.scalar.activation(o, ps[:, :], Relu, bias=bias)
                        elif bias is not None:
                            nc.vector.tensor_scalar(out=o, in0=ps[:, :],
                                                    scalar1=bias,
                                                    scalar2=None, op0=add)
                        else:
                            nc.vector.tensor_copy(out=o, in_=ps[:, :])

                pt = npo.tile([128, 1], f32, name="pt")
                for k0 in range(0, NQ, CH):
                    s1 = nsc.tile([128, CH], f32r, name="s1", tag="s1")
                    s2 = nsc.tile([128, CH], f32r, name="s2", tag="s2")
                    s3 = nsc.tile([128, CH], f32r, name="s3", tag="s3")
                    lay(s1, 0, [(W["a1"], pre3, k0, False)], Bt["a1"], True)
                    lay(s2, 0, [(W["a2"], s1, 0, True)], Bt["a2"], True)
                    lay(s3, 0, [(W["a3"], s2, 0, True)], Bt["a3"], False)
                    lay(s1, 0, [(W["c1x"], xt, k0, False),
                                (W["c1a"], s3, 0, True)], Bt["c1"], True)
                    lay(s2, 0, [(W["c2"], s1, 0, True)], Bt["c2"], True)
                    lay(nemb, k0, [(W["c3"], s2, 0, True)], Bt["c3"], False)
                    lay(s1, 0, [(W["n1x"], xt, k0, False),
                                (W["n1a"], nemb, k0, True)], Bt["n1"], True)
                    lay(s2, 0, [(W["n2"], s1, 0, True)], Bt["n2"], True)
                    lay(s3, 0, [(W["n3"], s2, 0, True)], Bt["n3"], False)
                    for (dl, s0, ln) in segs:
                        for q in range(4):
                            a = max(s0 - q * NQ, k0)
                            bb2 = min(s0 + ln - q * NQ, k0 + CH)
                            if bb2 > a:
                                nc.vector.tensor_reduce(
                                    pt[32 * q:32 * q + 16, 0:1],
                                    s3[32 * q:32 * q + 16, a - k0:bb2 - k0],
                                    X, add)
                                nc.vector.tensor_tensor(
                                    out=qd[32 * q:32 * q + 16, dl:dl + 1],
                                    in0=qd[32 * q:32 * q + 16, dl:dl + 1],
                                    in1=pt[32 * q:32 * q + 16, 0:1], op=add)

            p1_cm.__exit__(None, None, None)

            # dag-level chain (small, flat f32)
            with (
                tc.tile_pool(name="nd2", bufs=1) as nd2,
                tc.tile_pool(name="np2", bufs=2, space="PSUM") as np2,
            ):
                q1 = nd2.tile([16, NDMAX], f32, name="q1")
                q2 = nd2.tile([16, NDMAX], f32, name="q2")
                q3 = nd2.tile([16, NDMAX], f32, name="q3")
                nc.gpsimd.dma_start(out=q1[:, :], in_=qd[32:48, :])
                nc.gpsimd.dma_start(out=q2[:, :], in_=qd[64:80, :])
                nc.gpsimd.dma_start(out=q3[:, :], in_=qd[96:112, :])
                nc.vector.tensor_tensor(out=q1[:, :], in0=q1[:, :],
                                        in1=qd[0:16, :], op=add)
                nc.vector.tensor_tensor(out=q2[:, :], in0=q2[:, :],
                                        in1=q3[:, :], op=add)
                demb = nd2.tile([16, NDMAX], f32, name="demb")
                nc.vector.tensor_tensor(out=demb[:, :], in0=q1[:, :],
                                        in1=q2[:, :], op=add)

                def smm(dst, Wk, srcs, bias, relu, ncols, tag="sm"):
                    ps = np2.tile([32, max(ncols, 1)], f32, name=tag,
                                  tag="sm", bufs=1)
                    for i, (Wi, Si) in enumerate(zip(Wk, srcs)):
                        nc.tensor.matmul(ps[:dst.shape[0], :], Wi[:, :],
                                         Si[:, :ncols], start=(i == 0),
                                         stop=(i == len(Wk) - 1))
                    if relu:
                        nc.scalar.activation(dst[:, :ncols],
                                             ps[:dst.shape[0], :ncols], Relu,
                                             bias=bias)
                    elif bias is not None:
                        nc.vector.tensor_scalar(
                            out=dst[:, :ncols], in0=ps[:dst.shape[0], :ncols],
                            scalar1=bias, scalar2=None, op0=add)
                    else:
                        nc.vector.tensor_copy(out=dst[:, :ncols],
                                              in_=ps[:dst.shape[0], :ncols])

                dz1 = nd2.tile([32, NDMAX], f32, name="dz1")
                dz2 = nd2.tile([16, NDMAX], f32, name="dz2")
                dz = nd2.tile([16, NDMAX], f32, name="dz")
                smm(dz1, [W["d1"]], [demb], Bt["d1"], True, NDMAX)
                smm(dz2, [W["d2"]], [dz1], Bt["d2"], True, NDMAX)
                smm(dz, [W["d3"]], [dz2], Bt["d3"], False, NDMAX)
                gexp = nd2.tile([16, NDMAX], f32, name="gexp")
                nc.vector.memset(gexp[:, :], 0.0)
                gt = nd2.tile([16, max(len(c["osegs"]), 1)], f32, name="gt")
                for i, (o, dlo, nd, ndt) in enumerate(c["osegs"]):
                    nc.vector.tensor_reduce(gt[:, i:i + 1],
                                            dz[:, dlo:dlo + nd], X, add)
                    nc.vector.tensor_scalar(
                        out=gexp[:, dlo:dlo + nd],
                        in0=gt[:, i:i + 1].to_broadcast([16, nd]),
                        scalar1=1.0 / ndt, scalar2=None,
                        op0=mybir.AluOpType.mult)

                dcst = nd2.tile([128, NDMAX], f32, name="dcst")
                dc32 = nd2.tile([32, NDMAX], f32, name="dc32")
                smm(dc32, [W["s1d"], W["s1g"]], [demb, gexp], None, False,
                    NDMAX, tag="dc")
                nc.gpsimd.dma_start(out=dcst[0:32, :], in_=dc32[:, :])
                nc.gpsimd.dma_start(out=dcst[32:64, :], in_=dc32[:, :])
                nc.gpsimd.dma_start(out=dcst[64:96, :], in_=dc32[:, :])
                nc.gpsimd.dma_start(out=dcst[96:128, :], in_=dc32[:, :])

                # ============ pass 2: node scores ============
                def add_dag(ps, ka, kb):
                    for (dl, s0, ln) in segs:
                        for q in range(4):
                            a = max(s0 - q * NQ, ka)
                            bb2 = min(s0 + ln - q * NQ, kb)
                            if bb2 > a:
                                nc.vector.tensor_tensor(
                                    out=ps[32 * q:32 * q + 32, a - ka:bb2 - ka],
                                    in0=ps[32 * q:32 * q + 32, a - ka:bb2 - ka],
                                    in1=dcst[32 * q:32 * q + 32,
                                             dl:dl + 1].to_broadcast(
                                        [32, bb2 - a]),
                                    op=add)

                with (
                    tc.tile_pool(name="ns2", bufs=1) as ns2,
                    tc.tile_pool(name="nsp2", bufs=2, space="PSUM") as nsp2,
                ):
                    def lay2(dst, doff, ops, bias, relu, extra=None,
                             k0abs=0, copy_bias=None):
                        for k in range(0, CH, 512):
                            ps = nsp2.tile([128, 512], f32, name="ps2",
                                           tag="ps2")
                            n = len(ops)
                            for i, (Wi, Si, soff, rc) in enumerate(ops):
                                lh = Wi[:, :]
                                rh = Si[:, soff + k:soff + k + 512]
                                if rc:
                                    lh = lh.bitcast(f32r)
                                    rh = rh.bitcast(f32r)
                                nc.tensor.matmul(ps[:, :], lh, rh,
                                                 start=(i == 0),
                                                 stop=(i == n - 1))
                            if extra is not None:
                                extra(ps, k0abs + k, k0abs + k + 512)
                            o = dst[:, doff + k:doff + k + 512]
                            if copy_bias is not None:
                                nc.scalar.activation(o, ps[:, :], Copy,
                                                     bias=copy_bias)
                            elif relu:
                                nc.scalar.activation(o, ps[:, :], Relu,
                                                     bias=bias)

                    for k0 in range(0, NQ, CH):
                        t1 = ns2.tile([128, CH], f32r, name="t1", tag="t1")
                        t2 = ns2.tile([128, CH], f32r, name="t2", tag="t2")
                        nsb = ns2.tile([128, CH], f32, name="nsb", tag="nsb")
                        lay2(t1, 0, [(W["s1x"], xt, k0, False),
                                     (W["s1n"], nemb, k0, True)], Bt["s1"],
                             True, extra=add_dag, k0abs=k0)
                        lay2(t2, 0, [(W["s2"], t1, 0, True)], Bt["s2"], True)
                        lay2(nsb, 0, [(W["s3"], t2, 0, True)], None, False,
                             copy_bias=s3b)
                        nc.sync.dma_start(out=ns_d[:, k0:k0 + CH],
                                          in_=nsb[:, :])

                    # dag scores
                    dft = nd2.tile([3, NDMAX], bf, name="dft")
                    nc.sync.dma_start(out=dft[:, :], in_=dfeat_d[:, :])
                    dftf = nd2.tile([3, NDMAX], f32, name="dftf")
                    nc.vector.tensor_copy(out=dftf[:, :], in_=dft[:, :])
                    dsc = nd2.tile([32, NDMAX], f32, name="dsc")
                    smm(dsc, [W["g1f"], W["g1e"], W["g1g"]],
                        [dftf, demb, gexp], Bt["g1"], False, NDMAX, tag="dc")
                    wkt = nd2.tile([1, NWORK], bf, name="wkt")
                    nc.sync.dma_start(out=wkt[:, :], in_=wk_d[:, :])
                    wktf = nd2.tile([1, NWORK], f32, name="wktf")
                    nc.vector.tensor_copy(out=wktf[:, :], in_=wkt[:, :])
                    wkp = nsp2.tile([32, NWORK], f32, name="wkp", tag="sm2",
                                    bufs=1)
                    nc.tensor.matmul(wkp[:, :], W["g1w"][:, :], wktf[:, :],
                                     start=True, stop=True)
                    wkc = nd2.tile([32, NWORK], f32, name="wkc")
                    nc.vector.tensor_copy(out=wkc[:, :], in_=wkp[:, :])
                    ge1 = nd2.tile([32, NDS], f32, name="ge1")
                    nc.vector.tensor_tensor(
                        out=ge1[:, :],
                        in0=dsc[:, :, None].to_broadcast([32, NDMAX, NWORK]),
                        in1=wkc[:, None, :].to_broadcast([32, NDMAX, NWORK]),
                        op=add)
                    nc.scalar.activation(ge1[:, :], ge1[:, :], Relu)
                    ge2 = nd2.tile([16, NDS], f32, name="ge2")
                    for k in range(0, NDS, 512):
                        e = min(k + 512, NDS)
                        ps = nsp2.tile([16, 512], f32, name="g2p", tag="sm2",
                                       bufs=1)
                        nc.tensor.matmul(ps[:, :e - k], W["g2"][:, :],
                                         ge1[:, k:e], start=True, stop=True)
                        nc.scalar.activation(ge2[:, k:e], ps[:, :e - k],
                                             Relu, bias=Bt["g2"])
                    dsv = nd2.tile([1, NDS], f32, name="dsv")
                    for k in range(0, NDS, 512):
                        e = min(k + 512, NDS)
                        ps = nsp2.tile([1, 512], f32, name="g3p", tag="sm2",
                                       bufs=1)
                        nc.tensor.matmul(ps[:, :e - k], W["g3"][:, :],
                                         ge2[:, k:e], start=True, stop=True)
                        nc.scalar.activation(dsv[:, k:e], ps[:, :e - k],
                                             Copy, bias=g3b)
                    nc.sync.dma_start(out=ds_d[:, :], in_=dsv[:, :])

                    # num_nodes_per_obs / obs_indptr
                    ptrT = nd2.tile([1, G + 1], f32, name="ptrT")
                    nc.sync.dma_start(out=ptrT[:, :], in_=ptrT_d[:, :])
                    nnd = nd2.tile([1, G], f32, name="nnd")
                    nc.vector.tensor_tensor(out=nnd[:, :],
                                            in0=ptrT[:, 1:G + 1],
                                            in1=ptrT[:, 0:G],
                                            op=mybir.AluOpType.subtract)
                    nnov = nd2.tile([1, B], f32, name="nnov")
                    nc.vector.memset(nnov[:, :], 0.0)
                    for (o, glo, gn) in meta["gosegs"]:
                        if gn > 0:
                            nc.vector.tensor_reduce(
                                nnov[:, o:o + 1], nnd[:, glo:glo + gn], X,
                                add)
                    nc.sync.dma_start(out=nno_d[:, :], in_=nnov[:, :])
                    ndpo = nd2.tile([B, 1], bf, name="ndpo")
                    nc.sync.dma_start(out=ndpo[:, :], in_=ndpoT_d[:, :])
                    trit = nd2.tile([B, B + 1], bf, name="trit")
                    nc.sync.dma_start(out=trit[:, :], in_=tri_d[:, :])
                    oip_ps = nsp2.tile([B + 1, 1], f32, name="oipp",
                                       tag="sm2", bufs=1)
                    nc.tensor.matmul(oip_ps[:, :], trit[:, :], ndpo[:, :],
                                     start=True, stop=True)
                    oipv = nd2.tile([B + 1, 1], f32, name="oipv")
                    nc.vector.tensor_copy(out=oipv[:, :], in_=oip_ps[:, :])
                    nc.sync.dma_start(out=oip_d[:, :], in_=oipv[:, :])

            npo_cm.__exit__(None, None, None)

    return nc


# ----------------------------------------------------------------------
# entry point
# ----------------------------------------------------------------------

def _stripe(a, NQ):
    """[F<=32, NMAX] -> [128, NQ] quadrant-striped."""
    F = a.shape[0]
    out = np.zeros((128, NQ), np.float32)
    for q in range(4):
        out[32 * q:32 * q + F] = a[:, q * NQ:(q + 1) * NQ]
    return out


def kernel(x, edge_index, batch, ptr, num_dags_per_obs, params):
    x = np.asarray(x, np.float32)
    edge_index = np.asarray(edge_index)
    batch = np.asarray(batch)
    ptr = np.asarray(ptr)
    num_dags_per_obs = np.asarray(num_dags_per_obs)

    meta = _prep(x, edge_index, batch, ptr, num_dags_per_obs)
    w, b = _prep_weights(params)
    s3b, g3b = w.pop("s3b"), w.pop("g3b")
    b["b2row"] = np.tile(w.pop("b2row")[None, :], (128, 1))
    bsh = {k: (v.shape if v.ndim == 2 else (len(v), 1)) for k, v in b.items()}
    b = {k: (v if v.ndim == 2 else v[:, None]) for k, v in b.items()}
    bsh["b2row"] = b["b2row"].shape
    wsh = {k: v.shape for k, v in w.items()}

    NMAX, NQ, NDMAX, EMAX = meta["NMAX"], meta["NQ"], meta["NDMAX"], \
        meta["EMAX"]
    G, B = meta["G"], meta["B"]

    nc = _build(meta, wsh, bsh, s3b, g3b, meta["cores"][0])

    tri = np.zeros((B, B + 1), np.float32)
    for j in range(1, B + 1):
        tri[:j, j] = 1.0
    in_maps = []
    for cc in meta["cores"]:
        lo, n_c = cc["lo"], cc["n"]
        xsf = np.zeros((7, EMAX), np.float32)
        xsf[:6] = x[cc["src_p"]].T
        xsf[6] = 1.0
        # stream -> striped: seg s, quadrant q, col j = edge s*SEG+q*QE+j
        xs = np.zeros((128, EMAX // 4), np.float32)
        for q in range(4):
            cols = xsf.reshape(7, -1, 4, QE)[:, :, q, :].reshape(7, -1)
            xs[32 * q:32 * q + 7] = cols
        xtf = np.zeros((7, NMAX), np.float32)
        xtf[:6, :n_c] = x[lo:lo + n_c].T
        xtf[6] = 1.0
        dloc = cc["dloc_p"].reshape(-1, 128).T.astype(np.float32)
        d0, nd = cc["d0"], cc["nd"]
        dfeat = np.zeros((3, NDMAX), np.float32)
        if nd > 0:
            dfeat[:, :nd] = x[np.minimum(ptr[d0:d0 + nd].astype(np.int64),
                                         meta["N"] - 1), 0:3].T
        m = {"xsrc": xs.astype(BF), "dloc": dloc.astype(BF),
             "xt": _stripe(xtf, NQ).astype(BF), "dfeat": dfeat.astype(BF),
             "iota64": np.tile(np.arange(64, dtype=np.float32), (128, 1)).astype(BF),
             "wk": np.arange(NWORK, dtype=np.float32)[None].astype(BF),
             "ptrT": ptr.astype(np.float32)[None],
             "ndpoT": num_dags_per_obs.astype(np.float32)[:, None].astype(BF),
             "tri": tri.astype(BF)}
        for k2, v in w.items():
            m["w_" + k2] = v.astype(BF) if k2 in BF_W else v.astype(np.float32)
        for k2, v in b.items():
            m["b_" + k2] = v.astype(np.float32)
        in_maps.append(m)

    trace = bool(int(os.environ.get("KERNEL_TRACE", "0")))
    if trace:
        _install_ntff_hook()
    res = run_bass_kernel_spmd(nc, in_maps, core_ids=list(range(NCORES)),
                               trace=trace)
    if trace and res.exec_time_ns:
        print(f"HW exec time: {res.exec_time_ns} ns")

    N = meta["N"]
    node_scores = np.zeros(N, np.float32)
    dag_scores = np.zeros((G, NWORK), np.float32)
    for ci, cc in enumerate(meta["cores"]):
        r = res.results[ci]
        nsf = np.concatenate([r["ns"][32 * q:32 * q + 1] for q in range(4)],
                             axis=1)[0]
        node_scores[cc["lo"]:cc["lo"] + cc["n"]] = nsf[:cc["n"]]
        if cc["nd"] > 0:
            dag_scores[cc["d0"]:cc["d0"] + cc["nd"]] = \
                r["ds"][0, :cc["nd"] * NWORK].reshape(cc["nd"], NWORK)
    r0 = res.results[0]
    dt = num_dags_per_obs.dtype
    nno = np.round(r0["nno"][0]).astype(dt)
    oip = np.round(r0["oip"][:, 0]).astype(dt)
    return node_scores, dag_scores, nno, oip
